# revision 1
# baseline (speedup 1.0000x reference)
"""Multi-head cross-attention (B=2, N=1024, L=4096, D=1024, H=16) on 8 trn2
NeuronCores.

Sharding: batch x head-group data/tensor parallel. Core c handles batch
c//4 and heads 4*(c%4) .. 4*(c%4)+3 (weight columns sliced per head group,
Wo row-sliced; partial outputs summed on the host during unsharding).

Per-core device program (all matmuls in fp32r at full PE rate):
  qT/kT = W.T @ x.T    (channels on partitions, head pairs stacked 64+64)
  v     = x @ Wv       (keys on partitions) augmented with a ones column and
                       pre-multiplied by the pad-keep mask (this implements
                       the padding mask exactly: masked keys contribute to
                       neither numerator nor denominator)
  per (query-block, head-pair, keytile):
     sT[keys,q] = kT.T @ qT   (two row-paired K=64 matmuls)
     pT = exp(0.125 * sT)     (one ACT op over both heads' banks)
     oT_aug[65,q] += v_aug.T @ pT   (PSUM accumulation; row 64 = denominator)
  out_part = (oT/denom).T @ Wo_slice   (+ q/k/v biases via K=1 matmuls)
"""
import sys

sys.path.insert(0, "/opt/trn_rl_repo")

import numpy as np

import concourse.bass as bass
import concourse.tile as tile
from concourse import bacc, mybir
from concourse.bass_utils import run_bass_kernel_spmd

dt = mybir.dt
ts = bass.ts

B, N, L, D = 2, 1024, 4096, 1024
H, DH = 16, 64
HC = 4            # heads per core
CS = HC * DH      # 256 channel slice per core
SCALE = DH ** -0.5
N_CORES = 8
QB, KB = 2, 8     # query blocks of 512, key blocks of 512
DQC = 8           # contraction chunks of 128
KT = 32           # keytiles of 128

TRACE = False
LAST_EXEC_NS = None
_cache = {}


def _build():
    nc = bacc.Bacc("TRN2", target_bir_lowering=False, debug=False,
                   num_devices=N_CORES)

    xTq = nc.dram_tensor("xTq", [D, N], dt.float32, kind="ExternalInput").ap()
    xTkv = nc.dram_tensor("xTkv", [D, L], dt.float32, kind="ExternalInput").ap()
    wq = nc.dram_tensor("wq", [D, CS], dt.float32, kind="ExternalInput").ap()
    wk = nc.dram_tensor("wk", [D, CS], dt.float32, kind="ExternalInput").ap()
    wv = nc.dram_tensor("wv", [D, CS], dt.float32, kind="ExternalInput").ap()
    wo = nc.dram_tensor("wo", [CS, D], dt.float32, kind="ExternalInput").ap()
    bqv = nc.dram_tensor("bqv", [1, CS], dt.float32, kind="ExternalInput").ap()
    bkv = nc.dram_tensor("bkv", [1, CS], dt.float32, kind="ExternalInput").ap()
    bvv = nc.dram_tensor("bvv", [1, CS], dt.float32, kind="ExternalInput").ap()
    keep = nc.dram_tensor("keep", [128, KT, HC], dt.float32,
                          kind="ExternalInput").ap()
    out = nc.dram_tensor("out", [N, D], dt.float32, kind="ExternalOutput").ap()

    with tile.TileContext(nc) as tc:
        _emit(nc, tc, xTq, xTkv, wq, wk, wv, wo, bqv, bkv, bvv, keep, out)
    nc.compile()
    return nc


def _emit(nc, tc, xTq, xTkv, wq, wk, wv, wo, bqv, bkv, bvv, keep, out):
    import contextlib

    ctx = contextlib.ExitStack()
    with ctx:
        persist = ctx.enter_context(tc.tile_pool(name="persist", bufs=1))
        wstage = ctx.enter_context(tc.tile_pool(name="wstage", bufs=2))
        xstage = ctx.enter_context(tc.tile_pool(name="xstage", bufs=3))
        xr_pool = ctx.enter_context(tc.tile_pool(name="xr", bufs=10))
        pT_pool = ctx.enter_context(tc.tile_pool(name="pT", bufs=3))
        rb_pool = ctx.enter_context(tc.tile_pool(name="rbs", bufs=2))
        outsb_pool = ctx.enter_context(tc.tile_pool(name="outsb", bufs=3))
        psS = ctx.enter_context(tc.tile_pool(name="psS", bufs=2, space="PSUM"))
        psOA = ctx.enter_context(tc.tile_pool(name="psOA", bufs=1, space="PSUM"))
        psA_cm = tc.tile_pool(name="psA", bufs=1, space="PSUM")
        psA = psA_cm.__enter__()
        lp = nc.allow_low_precision(reason="fp32r attention internals")
        lp.__enter__()

        # ---- weight loading: one big DMA + one cast each -----------------
        def load_w3(name, src, d0):
            # src: DRAM [d0*128, F]; dst tile [128, d0, F] (chunk-major)
            f = wstage.tile([128, d0, src.shape[1]], dt.float32, tag="wstage",
                            name=f"{name}_f")
            nc.sync.dma_start(f[:], src.rearrange("(c p) n -> p c n", p=128))
            r = persist.tile([128, d0, src.shape[1]], dt.float32r, tag=name,
                             name=name)
            nc.vector.tensor_copy(r[:], f[:])
            return r

        def load_round(name, src, shape):
            f = wstage.tile(shape, dt.float32, tag="bstage", name=f"{name}_f")
            nc.sync.dma_start(f[:], src)
            r = persist.tile(shape, dt.float32r, tag=name, name=name)
            nc.vector.tensor_copy(r[:], f[:])
            return r

        wq_r = load_w3("wqr", wq, DQC)          # [128, 8, 256]
        bq_r = load_round("bqr", bqv, [1, CS])
        ones_f = wstage.tile([1, 512], dt.float32, tag="bstage", name="ones_f")
        nc.vector.memset(ones_f[:], 1.0)
        ones512_r = persist.tile([1, 512], dt.float32r, tag="o512", name="ones512_r")
        nc.vector.tensor_copy(ones512_r[:], ones_f[:])
        ones128_r = persist.tile([1, 128], dt.float32r, tag="o128", name="ones128_r")
        nc.vector.tensor_copy(ones128_r[:], ones_f[:, 0:128])

        # ---- persistent activation tiles --------------------------------
        qT_sb = [persist.tile([128, N], dt.float32r, tag=f"qT{cc}", name=f"qT{cc}")
                 for cc in range(2)]
        kT_sb = [[persist.tile([128, 512], dt.float32r, tag=f"kT{cc}_{kb}",
                               name=f"kT{cc}_{kb}") for kb in range(KB)]
                 for cc in range(2)]
        va_sb = [persist.tile([128, HC, 65], dt.float32r, tag=f"va{kt}",
                              name=f"va{kt}") for kt in range(KT)]
        onT_sb = [persist.tile([128, N], dt.float32r, tag=f"onT{cc}",
                               name=f"onT{cc}") for cc in range(2)]

        # ---- Q projection ----------------------------------------------
        for qb in range(QB):
            qp = psA.tile([128, 1024], dt.float32, tag="psA", name=f"qp{qb}")
            for dq in range(DQC):
                xf = xstage.tile([128, 512], dt.float32, tag="xs", name=f"xfq{qb}_{dq}")
                nc.sync.dma_start(xf[:], xTq[ts(dq, 128), ts(qb, 512)])
                xr = xr_pool.tile([128, 512], dt.float32r, tag="xr", name=f"xrq{qb}_{dq}")
                nc.vector.tensor_copy(xr[:], xf[:])
                for cc in range(2):
                    nc.tensor.matmul(qp[:, ts(cc, 512)], wq_r[:, dq, ts(cc, 128)],
                                     xr[:], start=(dq == 0), stop=False)
            for cc in range(2):
                nc.tensor.matmul(qp[:, ts(cc, 512)], bq_r[:, ts(cc, 128)],
                                 ones512_r[:], start=False, stop=True)
                nc.vector.tensor_copy(qT_sb[cc][:, ts(qb, 512)], qp[:, ts(cc, 512)])

        # remaining weights (DMA priority after the q-projection inputs)
        wk_r = load_w3("wkr", wk, DQC)
        wv_r = load_w3("wvr", wv, DQC)
        bk_r = load_round("bkr", bkv, [1, CS])
        bv_r = load_round("bvr", bvv, [1, CS])
        keep_f = persist.tile([128, KT, HC], dt.float32, tag="keepf", name="keep_f")
        nc.sync.dma_start(keep_f[:], keep)

        # ---- attention helpers ------------------------------------------
        oPs = {}

        def open_oP(qb, hp, pool, sfx):
            oPs[(qb, hp)] = [
                pool.tile([128, 512], dt.float32, tag=f"oP{h}{sfx}",
                          name=f"oP{qb}{hp}{h}")
                for h in range(2)
            ]

        def attn_kt(qb, hp, kt):
            kb, kti = kt // 4, kt % 4
            sp = psS.tile([128, 1024], dt.float32, tag="sp", name=f"sp{qb}{hp}{kt}")
            for h in range(2):
                nc.tensor.matmul(
                    sp[:, ts(h, 512)],
                    kT_sb[hp][kb][ts(h, 64), ts(kti, 128)],
                    qT_sb[hp][ts(h, 64), ts(qb, 512)],
                    start=True, stop=True,
                )
            pT = pT_pool.tile([128, 1024], dt.float32r, tag="pT", name=f"pT{qb}{hp}{kt}")
            nc.scalar.activation(pT[:], sp[:], mybir.ActivationFunctionType.Exp,
                                 scale=float(SCALE))
            oP = oPs[(qb, hp)]
            for h in range(2):
                nc.tensor.matmul(
                    oP[h][0:65, :], va_sb[kt][:, hp * 2 + h, :], pT[:, ts(h, 512)],
                    start=(kt == 0), stop=(kt == KT - 1),
                )

        def attn_norm(qb, hp):
            oP = oPs.pop((qb, hp))
            for h in range(2):
                den = rb_pool.tile([1, 512], dt.float32, tag="den",
                                   name=f"den{qb}{hp}{h}")
                nc.vector.tensor_copy(den[:], oP[h][64:65, :])
                rdf = rb_pool.tile([1, 512], dt.float32, tag="rdf",
                                   name=f"rdf{qb}{hp}{h}")
                # approx_fast needs an SBUF source (PSUM source returns garbage)
                nc.vector.reciprocal_approx_fast(rdf[:], den[:])
                rd = rb_pool.tile([1, 512], dt.float32r, tag="rd",
                                  name=f"rd{qb}{hp}{h}")
                nc.vector.tensor_copy(rd[:], rdf[:])
                rb = psS.tile([128, 512], dt.float32, tag="sp", name=f"rb{qb}{hp}{h}")
                nc.tensor.matmul(rb[:, :], ones128_r[:], rd[:], start=True, stop=True)
                rb_sb = rb_pool.tile([128, 512], dt.float32, tag="rbs",
                                     name=f"rbs{qb}{hp}{h}")
                nc.vector.tensor_copy(rb_sb[:], rb[:])
                nc.vector.tensor_mul(onT_sb[hp][ts(h, 64), ts(qb, 512)],
                                     oP[h][0:64, :], rb_sb[0:64, :])

        # ---- K/V projections interleaved with attention on (qb0, hp0) ---
        open_oP(0, 0, psOA, "a")
        for kb in range(KB):
            kp = psA.tile([128, 1024], dt.float32, tag="psA", name=f"kp{kb}")
            xrs = []
            for dq in range(DQC):
                xf = xstage.tile([128, 512], dt.float32, tag="xs", name=f"xfk{kb}_{dq}")
                nc.sync.dma_start(xf[:], xTkv[ts(dq, 128), ts(kb, 512)])
                xr = xr_pool.tile([128, 512], dt.float32r, tag="xr", name=f"xrk{kb}_{dq}")
                nc.vector.tensor_copy(xr[:], xf[:])
                xrs.append(xr)
                for cc in range(2):
                    nc.tensor.matmul(kp[:, ts(cc, 512)], wk_r[:, dq, ts(cc, 128)],
                                     xr[:], start=(dq == 0), stop=False)
            for cc in range(2):
                nc.tensor.matmul(kp[:, ts(cc, 512)], bk_r[:, ts(cc, 128)],
                                 ones512_r[:], start=False, stop=True)
                nc.vector.tensor_copy(kT_sb[cc][kb][:], kp[:, ts(cc, 512)])

            vp = psA.tile([128, 1024], dt.float32, tag="psA", name=f"vp{kb}")
            for dq in range(DQC):
                for t in range(4):
                    # start clears has_written for the whole 2KB psum bank, so
                    # only the first matmul touching each bank may set it
                    nc.tensor.matmul(vp[:, ts(t, 256)], xrs[dq][:, ts(t, 128)],
                                     wv_r[:, dq, :],
                                     start=(dq == 0 and t % 2 == 0), stop=False)
            for t in range(4):
                nc.tensor.matmul(vp[:, ts(t, 256)], ones128_r[:], bv_r[:],
                                 start=False, stop=True)
            for t in range(4):
                kt = kb * 4 + t
                va = va_sb[kt]
                src = vp[:, ts(t, 256)].rearrange("p (h c) -> p h c", h=HC)
                nc.vector.tensor_scalar_mul(va[:, :, 0:64], src,
                                            keep_f[:, kt, 0:1])
                nc.vector.tensor_copy(va[:, :, 64:65], keep_f[:, kt, :])

            for t in range(4):
                attn_kt(0, 0, kb * 4 + t)

        # projections done: release psA's 2 banks, open the second oP pool
        psA_cm.__exit__(None, None, None)
        psOB = ctx.enter_context(tc.tile_pool(name="psOB", bufs=1, space="PSUM"))

        wo_r = load_w3("wor", wo, 2)            # [128, 2, 1024]

        attn_norm(0, 0)

        # ---- remaining attention combos (alternating psum pools) --------
        for i, (qb, hp) in enumerate([(0, 1), (1, 0), (1, 1)]):
            pool, sfx = (psOB, "b") if i % 2 == 0 else (psOA, "a")
            open_oP(qb, hp, pool, sfx)
            for kt in range(KT):
                attn_kt(qb, hp, kt)
            attn_norm(qb, hp)

        # ---- output projection ------------------------------------------
        for qt in range(8):
            for eb in range(2):
                pool, sfx = (psOB, "b") if (qt * 2 + eb) % 2 == 0 else (psOA, "a")
                op = pool.tile([128, 512], dt.float32, tag=f"oP0{sfx}",
                               name=f"op{qt}_{eb}")
                for cc in range(2):
                    nc.tensor.matmul(op[:, :], onT_sb[cc][:, ts(qt, 128)],
                                     wo_r[:, cc, ts(eb, 512)],
                                     start=(cc == 0), stop=(cc == 1))
                osb = outsb_pool.tile([128, 512], dt.float32, tag="osb",
                                      name=f"osb{qt}_{eb}")
                nc.vector.tensor_copy(osb[:], op[:])
                nc.sync.dma_start(out[ts(qt, 128), ts(eb, 512)], osb[:])

        lp.__exit__(None, None, None)


def kernel(x_q, x_kv, pad_mask, Wq, bq, Wk, bk, Wv, bv, Wo, bo):
    global LAST_EXEC_NS
    x_q = np.asarray(x_q, np.float32)
    x_kv = np.asarray(x_kv, np.float32)
    pad_mask = np.asarray(pad_mask)
    Wq, bq = np.asarray(Wq, np.float32), np.asarray(bq, np.float32)
    Wk, bk = np.asarray(Wk, np.float32), np.asarray(bk, np.float32)
    Wv, bv = np.asarray(Wv, np.float32), np.asarray(bv, np.float32)
    Wo, bo = np.asarray(Wo, np.float32), np.asarray(bo, np.float32)

    if "nc" not in _cache:
        _cache["nc"] = _build()
    nc = _cache["nc"]

    xTq = [np.ascontiguousarray(x_q[b].T) for b in range(B)]
    xTkv = [np.ascontiguousarray(x_kv[b].T) for b in range(B)]
    keepm = []
    for b in range(B):
        k01 = (~pad_mask[b]).astype(np.float32)          # (L,) 1=keep
        k4 = np.repeat(k01[:, None], HC, axis=1)          # (L, HC)
        keepm.append(np.ascontiguousarray(
            k4.reshape(KT, 128, HC).transpose(1, 0, 2)))  # (128, KT, HC)

    in_maps = []
    for c in range(N_CORES):
        b, g = c // 4, c % 4
        hs = g * CS
        in_maps.append({
            "xTq": xTq[b],
            "xTkv": xTkv[b],
            "wq": np.ascontiguousarray(Wq[:, hs:hs + CS]),
            "wk": np.ascontiguousarray(Wk[:, hs:hs + CS]),
            "wv": np.ascontiguousarray(Wv[:, hs:hs + CS]),
            "wo": np.ascontiguousarray(Wo[hs:hs + CS, :]),
            "bqv": np.ascontiguousarray(bq[hs:hs + CS][None, :]),
            "bkv": np.ascontiguousarray(bk[hs:hs + CS][None, :]),
            "bvv": np.ascontiguousarray(bv[hs:hs + CS][None, :]),
            "keep": keepm[b],
        })

    res = run_bass_kernel_spmd(nc, in_maps, list(range(N_CORES)), trace=TRACE)
    LAST_EXEC_NS = res.exec_time_ns

    outp = np.zeros((B, N, D), np.float32)
    for c in range(N_CORES):
        outp[c // 4] += res.results[c]["out"]
    outp += bo
    return outp



# revision 8
# speedup vs baseline: 1.0275x; 1.0275x over previous
"""Multi-head cross-attention (B=2, N=1024, L=4096, D=1024, H=16) on 8 trn2
NeuronCores.

Sharding: batch x head-group data/tensor parallel. Core c handles batch
c//4 and heads 4*(c%4) .. 4*(c%4)+3 (weight columns sliced per head group,
Wo row-sliced; partial outputs summed on the host during unsharding).

Math simplifications vs the reference (exact, not approximations):
  - bk dropped: scores shift per-query by (q+bq)@bk, softmax-invariant
    (numerator and denominator share the exp(q@bk) factor).
  - bv dropped on device: softmax rows sum to 1, so the bias contributes
    bv @ Wo, a constant row added on the host together with bo.
  - softmax scale folded into Wq and bq on the host.

Per-core device program (all matmuls fp32r; DMA loads feed fp32r tiles
directly -- fp32r is bitwise fp32 -- so there are no cast copies):
  qT = W.T @ x.T       (channels on partitions, head pairs stacked 64+64)
  kT likewise per 512-key block; v = x @ Wv (keys on partitions),
    augmented with a ones column and pre-multiplied by the pad-keep mask
    (masked keys contribute to neither numerator nor denominator)
  per combo (query-block, head) over keytile PAIRS:
     sT[keys, q] = kT.T @ qT  for kt and kt+1 into one 2-bank PSUM tile
     pT = exp(sT)             (one ACT op covering both keytiles)
     oT_aug[65,q] += va.T @ pT   (two accumulating matmuls; row 64 = denom)
  out_part = (oT/denom).T @ Wo_slice

Scheduling: the PE p-state only reaches 2.4GHz after ~3us of unbroken
execution, so the emission order is built around PE continuity:
  - single-head combos make per-exp PE work (4 matmuls) exceed the ACT
    latency, so the score->exp->attnV chain never starves the PE
  - projections and head-0 attention interleave with a one-kb lag so
    PSUM->SBUF drains are never immediately ahead of their consumers
  - each combo's normalization (DVE reciprocal chain + PE broadcast) is
    emitted a few keytiles INTO the next combo
  - the reciprocal broadcast matmuls target the projection staging pool
    (kept alive), so they never contend with score/attnV PSUM tiles
"""
import sys

sys.path.insert(0, "/opt/trn_rl_repo")

import numpy as np

import concourse.bass as bass
import concourse.tile as tile
from concourse import bacc, mybir
from concourse.bass_utils import run_bass_kernel_spmd

dt = mybir.dt
ts = bass.ts

B, N, L, D = 2, 1024, 4096, 1024
H, DH = 16, 64
HC = 4            # heads per core
CS = HC * DH      # 256 channel slice per core
SCALE = DH ** -0.5
N_CORES = 8
QB, KB = 2, 8     # query blocks of 512, key blocks of 512
DQC = 8           # contraction chunks of 128
KT = 32           # keytiles of 128

TRACE = False
LAST_EXEC_NS = None
_cache = {}


def _build():
    nc = bacc.Bacc("TRN2", target_bir_lowering=False, debug=False,
                   num_devices=N_CORES)

    f32r = dt.float32r
    xTq = nc.dram_tensor("xTq", [D, N], f32r, kind="ExternalInput").ap()
    xTkv = nc.dram_tensor("xTkv", [D, L], f32r, kind="ExternalInput").ap()
    wq = nc.dram_tensor("wq", [D, CS], f32r, kind="ExternalInput").ap()
    wk = nc.dram_tensor("wk", [D, CS], f32r, kind="ExternalInput").ap()
    wv = nc.dram_tensor("wv", [D, CS], f32r, kind="ExternalInput").ap()
    wo = nc.dram_tensor("wo", [CS, D], f32r, kind="ExternalInput").ap()
    bqp = nc.dram_tensor("bqp", [128, 2], dt.float32, kind="ExternalInput").ap()
    keep = nc.dram_tensor("keep", [128, KT, HC], dt.float32,
                          kind="ExternalInput").ap()
    out = nc.dram_tensor("out", [N, D], dt.float32, kind="ExternalOutput").ap()

    with tile.TileContext(nc) as tc:
        _emit(nc, tc, xTq, xTkv, wq, wk, wv, wo, bqp, keep, out)
    nc.compile()
    return nc


def _emit(nc, tc, xTq, xTkv, wq, wk, wv, wo, bqp, keep, out):
    import contextlib

    ctx = contextlib.ExitStack()
    with ctx:
        persist = ctx.enter_context(tc.tile_pool(name="persist", bufs=1))
        xr_pool = ctx.enter_context(tc.tile_pool(name="xr", bufs=8))
        pT_pool = ctx.enter_context(tc.tile_pool(name="pT", bufs=3))
        rb_pool = ctx.enter_context(tc.tile_pool(name="rbs", bufs=2))
        outsb_pool = ctx.enter_context(tc.tile_pool(name="outsb", bufs=3))
        # PSUM: psA ([128,512]x2 = 2 banks) stages K/V projections in phase B
        # and reciprocal broadcasts afterwards; psS ([128,1024]x2 = 4 banks)
        # holds Q-projection then score/exp tiles; psO ([128,512]x2 = 2
        # banks) ping-pongs per-combo attnV accumulators.
        psA = ctx.enter_context(tc.tile_pool(name="psA", bufs=2, space="PSUM"))
        psS = ctx.enter_context(tc.tile_pool(name="psS", bufs=2, space="PSUM"))
        psO = ctx.enter_context(tc.tile_pool(name="psO", bufs=2, space="PSUM"))
        lp = nc.allow_low_precision(reason="fp32r attention internals")
        lp.__enter__()

        # ---- weight loading: one DMA straight into an fp32r tile ---------
        def load_w3(name, src, d0):
            # src: DRAM [d0*128, F]; dst tile [128, d0, F] (chunk-major)
            r = persist.tile([128, d0, src.shape[1]], dt.float32r, tag=name,
                             name=name)
            nc.sync.dma_start(r[:], src.rearrange("(c p) n -> p c n", p=128))
            return r

        wq_r = load_w3("wqr", wq, DQC)          # [128, 8, 256]
        bq_sb = persist.tile([128, 2], dt.float32, tag="bqp", name="bq_sb")
        nc.sync.dma_start(bq_sb[:], bqp)
        ones128_f = persist.tile([1, 128], dt.float32, tag="o128",
                                 name="ones128_f")
        nc.vector.memset(ones128_f[:], 1.0)
        ones128_r = ones128_f[:].bitcast(dt.float32r)

        # ---- persistent activation tiles --------------------------------
        qT_sb = [persist.tile([128, N], dt.float32r, tag=f"qT{cc}", name=f"qT{cc}")
                 for cc in range(2)]
        kT_sb = [[persist.tile([128, 512], dt.float32r, tag=f"kT{cc}_{kb}",
                               name=f"kT{cc}_{kb}") for kb in range(KB)]
                 for cc in range(2)]
        va_sb = [persist.tile([128, HC, 65], dt.float32r, tag=f"va{kt}",
                              name=f"va{kt}") for kt in range(KT)]
        onT_sb = [persist.tile([128, N], dt.float32r, tag=f"onT{cc}",
                               name=f"onT{cc}") for cc in range(2)]

        # ---- Q projection (into psS while attention hasn't started) -----
        xq_r = []
        for dq in range(DQC):
            xr = xr_pool.tile([128, N], dt.float32r, tag="xr", name=f"xq{dq}")
            nc.sync.dma_start(xr[:], xTq[ts(dq, 128), :])
            xq_r.append(xr)
        for cc in range(2):
            qp = psS.tile([128, N], dt.float32, tag="sp", name=f"qp{cc}")
            for qb in range(QB):
                for dq in range(DQC):
                    nc.tensor.matmul(qp[:, ts(qb, 512)],
                                     wq_r[:, dq, ts(cc, 128)],
                                     xq_r[dq][:, ts(qb, 512)],
                                     start=(dq == 0), stop=(dq == DQC - 1))
            # drain + bias in one fused DVE op (per-partition scalar add)
            nc.vector.tensor_scalar_add(qT_sb[cc][:], qp[:], bq_sb[:, cc:cc + 1])

        # remaining weights (DMA priority after the q-projection inputs)
        wk_r = load_w3("wkr", wk, DQC)
        wv_r = load_w3("wvr", wv, DQC)
        keep_f = persist.tile([128, KT, HC], dt.float32, tag="keepf",
                              name="keep_f")
        nc.sync.dma_start(keep_f[:], keep)

        # ---- attention helpers ------------------------------------------
        # combo = (qb, h): query block x single head, over keytile pairs
        oPs = {}

        def open_oP(qb, h):
            oPs[(qb, h)] = psO.tile([128, 512], dt.float32, tag="oP",
                                    name=f"oP{qb}{h}")

        def attn_pair(qb, h, kt0):
            hp, hh = h // 2, h % 2
            sp = psS.tile([128, 1024], dt.float32, tag="sp",
                          name=f"sp{qb}{h}{kt0}")
            for j in range(2):
                kt = kt0 + j
                kb, kti = kt // 4, kt % 4
                nc.tensor.matmul(
                    sp[:, ts(j, 512)],
                    kT_sb[hp][kb][ts(hh, 64), ts(kti, 128)],
                    qT_sb[hp][ts(hh, 64), ts(qb, 512)],
                    start=True, stop=True,
                )
            pT = pT_pool.tile([128, 1024], dt.float32r, tag="pT",
                              name=f"pT{qb}{h}{kt0}")
            nc.scalar.activation(pT[:], sp[:], mybir.ActivationFunctionType.Exp)
            oP = oPs[(qb, h)]
            for j in range(2):
                kt = kt0 + j
                nc.tensor.matmul(
                    oP[0:65, :], va_sb[kt][:, h, :], pT[:, ts(j, 512)],
                    start=(kt == 0), stop=(kt == KT - 1),
                )

        def attn_norm(qb, h):
            hp, hh = h // 2, h % 2
            oP = oPs.pop((qb, h))
            den = rb_pool.tile([1, 512], dt.float32, tag="den",
                               name=f"den{qb}{h}")
            nc.vector.tensor_copy(den[:], oP[64:65, :])
            # approx_fast needs an SBUF source (PSUM source returns garbage)
            rdf = rb_pool.tile([1, 512], dt.float32, tag="rdf",
                               name=f"rdf{qb}{h}")
            nc.vector.reciprocal_approx_fast(rdf[:], den[:])
            rd = rb_pool.tile([1, 512], dt.float32r, tag="rd",
                              name=f"rd{qb}{h}")
            nc.vector.tensor_copy(rd[:], rdf[:])
            rb = psA.tile([128, 512], dt.float32, tag="psA", name=f"rb{qb}{h}")
            nc.tensor.matmul(rb[:, :], ones128_r, rd[:], start=True, stop=True)
            rb_sb = rb_pool.tile([128, 512], dt.float32, tag="rbs",
                                 name=f"rbs{qb}{h}")
            nc.vector.tensor_copy(rb_sb[:], rb[:])
            nc.vector.tensor_mul(onT_sb[hp][ts(hh, 64), ts(qb, 512)],
                                 oP[0:64, :], rb_sb[0:64, :])

        # ---- K/V projections interleaved with head-0 attention ----------
        # One-kb lag: attention for keytiles of block kb-1 is emitted while
        # block kb is being projected, so PSUM->SBUF drains (DVE) are never
        # immediately ahead of the matmuls that consume them.
        open_oP(0, 0)
        open_oP(1, 0)

        def proj_kb(kb):
            xrs = []
            for dq in range(DQC):
                xr = xr_pool.tile([128, 512], dt.float32r, tag="xr",
                                  name=f"xk{kb}_{dq}")
                nc.sync.dma_start(xr[:], xTkv[ts(dq, 128), ts(kb, 512)])
                xrs.append(xr)
            for cc in range(2):
                kp = psA.tile([128, 512], dt.float32, tag="psA", name=f"kp{kb}{cc}")
                for dq in range(DQC):
                    nc.tensor.matmul(kp[:], wk_r[:, dq, ts(cc, 128)], xrs[dq][:],
                                     start=(dq == 0), stop=(dq == DQC - 1))
                nc.vector.tensor_copy(kT_sb[cc][kb][:], kp[:])
            for half in range(2):
                vp = psA.tile([128, 512], dt.float32, tag="psA",
                              name=f"vp{kb}{half}")
                for dq in range(DQC):
                    for t2 in range(2):
                        t = half * 2 + t2
                        # start clears pending-write state for the whole 2KB
                        # psum bank: only its first matmul may set it
                        nc.tensor.matmul(vp[:, ts(t2, 256)],
                                         xrs[dq][:, ts(t, 128)],
                                         wv_r[:, dq, :],
                                         start=(dq == 0 and t2 == 0),
                                         stop=(dq == DQC - 1))
                for t2 in range(2):
                    t = half * 2 + t2
                    kt = kb * 4 + t
                    va = va_sb[kt]
                    src = vp[:, ts(t2, 256)].rearrange("p (h c) -> p h c", h=HC)
                    nc.vector.tensor_scalar_mul(va[:, :, 0:64], src,
                                                keep_f[:, kt, 0:1])
                    nc.vector.tensor_copy(va[:, :, 64:65], keep_f[:, kt, :])

        def b_attn(kb):
            # 4 keytiles of block kb for head 0, both query blocks
            for qb in range(QB):
                for p in range(2):
                    attn_pair(qb, 0, kb * 4 + 2 * p)

        for kb in range(KB):
            proj_kb(kb)
            if kb >= 1:
                b_attn(kb - 1)
        b_attn(KB - 1)

        wo_r = load_w3("wor", wo, 2)            # [128, 2, 1024]

        # ---- remaining combos -------------------------------------------
        # psO has 2 buffers, so combo c_i reuses c_{i-2}'s accumulator:
        # norm(c_{i-2}) is emitted right BEFORE c_i opens, keeping its
        # broadcast matmul ahead of c_i's attnV in the in-order PE queue
        # (emitting it later would deadlock PE <-> DVE).
        combos = [(qb, h) for h in range(1, HC) for qb in range(QB)]
        pending = [(0, 0), (1, 0)]
        for qb, h in combos:
            if len(pending) == 2:
                attn_norm(*pending.pop(0))
            open_oP(qb, h)
            for p in range(KT // 2):
                attn_pair(qb, h, 2 * p)
            pending.append((qb, h))
        for c in pending:
            attn_norm(*c)

        # ---- output projection ------------------------------------------
        for qt in range(8):
            for eb in range(2):
                op = psO.tile([128, 512], dt.float32, tag="oP",
                              name=f"op{qt}_{eb}")
                for cc in range(2):
                    nc.tensor.matmul(op[:, :], onT_sb[cc][:, ts(qt, 128)],
                                     wo_r[:, cc, ts(eb, 512)],
                                     start=(cc == 0), stop=(cc == 1))
                osb = outsb_pool.tile([128, 512], dt.float32, tag="osb",
                                      name=f"osb{qt}_{eb}")
                nc.vector.tensor_copy(osb[:], op[:])
                nc.sync.dma_start(out[ts(qt, 128), ts(eb, 512)], osb[:])

        lp.__exit__(None, None, None)


def kernel(x_q, x_kv, pad_mask, Wq, bq, Wk, bk, Wv, bv, Wo, bo):
    global LAST_EXEC_NS
    x_q = np.asarray(x_q, np.float32)
    x_kv = np.asarray(x_kv, np.float32)
    pad_mask = np.asarray(pad_mask)
    Wq, bq = np.asarray(Wq, np.float32), np.asarray(bq, np.float32)
    Wk, bk = np.asarray(Wk, np.float32), np.asarray(bk, np.float32)
    Wv, bv = np.asarray(Wv, np.float32), np.asarray(bv, np.float32)
    Wo, bo = np.asarray(Wo, np.float32), np.asarray(bo, np.float32)

    if "nc" not in _cache:
        _cache["nc"] = _build()
    nc = _cache["nc"]

    Wq_s = (Wq * SCALE).astype(np.float32)
    bq_s = (bq * SCALE).astype(np.float32)

    xTq = [np.ascontiguousarray(x_q[b].T) for b in range(B)]
    xTkv = [np.ascontiguousarray(x_kv[b].T) for b in range(B)]
    keepm = []
    for b in range(B):
        k01 = (~pad_mask[b]).astype(np.float32)          # (L,) 1=keep
        k4 = np.repeat(k01[:, None], HC, axis=1)          # (L, HC)
        keepm.append(np.ascontiguousarray(
            k4.reshape(KT, 128, HC).transpose(1, 0, 2)))  # (128, KT, HC)

    in_maps = []
    for c in range(N_CORES):
        b, g = c // 4, c % 4
        hs = g * CS
        in_maps.append({
            "xTq": xTq[b],
            "xTkv": xTkv[b],
            "wq": np.ascontiguousarray(Wq_s[:, hs:hs + CS]),
            "wk": np.ascontiguousarray(Wk[:, hs:hs + CS]),
            "wv": np.ascontiguousarray(Wv[:, hs:hs + CS]),
            "wo": np.ascontiguousarray(Wo[hs:hs + CS, :]),
            "bqp": np.ascontiguousarray(
                bq_s[hs:hs + CS].reshape(2, 128).T),      # [128, 2] cc-major
            "keep": keepm[b],
        })

    res = run_bass_kernel_spmd(nc, in_maps, list(range(N_CORES)), trace=TRACE)
    LAST_EXEC_NS = res.exec_time_ns

    outp = np.zeros((B, N, D), np.float32)
    for c in range(N_CORES):
        outp[c // 4] += res.results[c]["out"]
    outp += bo + bv @ Wo
    return outp


# revision 14
# speedup vs baseline: 1.3591x; 1.3228x over previous
"""Multi-head cross-attention (B=2, N=1024, L=4096, D=1024, H=16) on 8 trn2
NeuronCores.

Sharding: batch x head-group data/tensor parallel. Core c handles batch
c//4 and heads 4*(c%4) .. 4*(c%4)+3 (weight columns sliced per head group,
Wo row-sliced; partial outputs summed on the host during unsharding).

Math simplifications vs the reference (exact, not approximations):
  - bk dropped: scores shift per-query by (q+bq)@bk, softmax-invariant.
  - bv dropped on device: softmax rows sum to 1, so the bias contributes
    bv @ Wo, a constant row added on the host together with bo.
  - softmax scale folded into Wq and bq on the host.

Per-core device program. Projections run in fp32r (bitwise fp32, full
rate); the attention inner loop (scores, softmax weights, V) runs in
bf16 -- measured end-to-end error 1.8e-3, and the halved SBUF traffic
keeps the power governor from down-clocking the PE:
  qT = W.T @ x.T       (channels on partitions, head pairs stacked 64+64)
  kT likewise per 512-key block; v = x @ Wv (keys on partitions),
    augmented with a ones column and pre-multiplied by the pad-keep mask
  per (query-block, head-pair, keytile):
     sT[keys,q] = kT.T @ qT   (two row-paired K=64 bf16 matmuls)
     pT = exp(sT) in bf16     (one ACT op over both heads' banks)
     oT_aug[65,q] += va.T @ pT   (PSUM accumulation; row 64 = denominator)
  out_part = (oT/denom).T @ Wo_slice

Scheduling: PE clock is HAM-gated (1.2 GHz cold, 2.4 GHz after ~3.4us of
sustained activity) and power-throttled under sustained full draw, so the
emission order keeps the PE fed without idling ACT:
  - projections and combo (0,0) attention interleave with K/V staging in
    single-bank PSUM tiles (no drain ever blocks the next matmul)
  - in the attention-only phase a third score buffer (in the banks freed
    by the projection pool) lets two exps stay in flight so the ACT
    stream -- the throughput floor -- never starves
"""
import sys

sys.path.insert(0, "/opt/trn_rl_repo")

import numpy as np

import concourse.bass as bass
import concourse.tile as tile
from concourse import bacc, mybir
from concourse.bass_utils import run_bass_kernel_spmd

dt = mybir.dt
ts = bass.ts

B, N, L, D = 2, 1024, 4096, 1024
H, DH = 16, 64
HC = 4            # heads per core
CS = HC * DH      # 256 channel slice per core
SCALE = DH ** -0.5
N_CORES = 8
QB, KB = 2, 8     # query blocks of 512, key blocks of 512
DQC = 8           # contraction chunks of 128
KT = 32           # keytiles of 128

TRACE = False
LAST_EXEC_NS = None
_cache = {}


def _build():
    nc = bacc.Bacc("TRN2", target_bir_lowering=False, debug=False,
                   num_devices=N_CORES)

    f32r = dt.float32r
    xTq = nc.dram_tensor("xTq", [D, N], f32r, kind="ExternalInput").ap()
    xTkv = nc.dram_tensor("xTkv", [D, L], f32r, kind="ExternalInput").ap()
    wq = nc.dram_tensor("wq", [D, CS], f32r, kind="ExternalInput").ap()
    wk = nc.dram_tensor("wk", [D, CS], f32r, kind="ExternalInput").ap()
    wv = nc.dram_tensor("wv", [D, CS], f32r, kind="ExternalInput").ap()
    wo = nc.dram_tensor("wo", [CS, D], f32r, kind="ExternalInput").ap()
    bqp = nc.dram_tensor("bqp", [128, 2], dt.float32, kind="ExternalInput").ap()
    keep = nc.dram_tensor("keep", [128, KT, HC], dt.float32,
                          kind="ExternalInput").ap()
    out = nc.dram_tensor("out", [N, D], dt.float32, kind="ExternalOutput").ap()

    with tile.TileContext(nc) as tc:
        _emit(nc, tc, xTq, xTkv, wq, wk, wv, wo, bqp, keep, out)
    nc.compile()
    return nc


def _emit(nc, tc, xTq, xTkv, wq, wk, wv, wo, bqp, keep, out):
    import contextlib

    ctx = contextlib.ExitStack()
    with ctx:
        persist = ctx.enter_context(tc.tile_pool(name="persist", bufs=1))
        xr_pool = ctx.enter_context(tc.tile_pool(name="xr", bufs=8))
        pT_pool = ctx.enter_context(tc.tile_pool(name="pT", bufs=4))
        rb_pool = ctx.enter_context(tc.tile_pool(name="rbs", bufs=2))
        outsb_pool = ctx.enter_context(tc.tile_pool(name="outsb", bufs=3))
        psS = ctx.enter_context(tc.tile_pool(name="psS", bufs=2, space="PSUM"))
        psO = ctx.enter_context(tc.tile_pool(name="psO", bufs=2, space="PSUM"))
        psA_cm = tc.tile_pool(name="psA", bufs=2, space="PSUM")
        psA = psA_cm.__enter__()
        lp = nc.allow_low_precision(reason="fp32r/bf16 attention internals")
        lp.__enter__()

        # ---- weight loading: one DMA straight into an fp32r tile ---------
        def load_w3(name, src, d0):
            r = persist.tile([128, d0, src.shape[1]], dt.float32r, tag=name,
                             name=name)
            nc.sync.dma_start(r[:], src.rearrange("(c p) n -> p c n", p=128))
            return r

        wq_r = load_w3("wqr", wq, DQC)          # [128, 8, 256]
        bq_sb = persist.tile([128, 2], dt.float32, tag="bqp", name="bq_sb")
        nc.sync.dma_start(bq_sb[:], bqp)
        ones128_f = persist.tile([1, 128], dt.float32, tag="o128",
                                 name="ones128_f")
        nc.vector.memset(ones128_f[:], 1.0)
        ones128_r = ones128_f[:].bitcast(dt.float32r)

        # ---- persistent activation tiles (attention operands in bf16) ---
        qT_sb = [persist.tile([128, N], dt.bfloat16, tag=f"qT{cc}", name=f"qT{cc}")
                 for cc in range(2)]
        kT_sb = [[persist.tile([128, 512], dt.bfloat16, tag=f"kT{cc}_{kb}",
                               name=f"kT{cc}_{kb}") for kb in range(KB)]
                 for cc in range(2)]
        va_sb = [persist.tile([128, HC, 65], dt.bfloat16, tag=f"va{kt}",
                              name=f"va{kt}") for kt in range(KT)]
        onT_sb = [persist.tile([128, N], dt.float32r, tag=f"onT{cc}",
                               name=f"onT{cc}") for cc in range(2)]

        # ---- Q projection (into psS while attention hasn't started) -----
        xq_r = []
        for dq in range(DQC):
            xr = xr_pool.tile([128, N], dt.float32r, tag="xr", name=f"xq{dq}")
            nc.sync.dma_start(xr[:], xTq[ts(dq, 128), :])
            xq_r.append(xr)
        for cc in range(2):
            qp = psS.tile([128, N], dt.float32, tag="sp", name=f"qp{cc}")
            for qb in range(QB):
                for dq in range(DQC):
                    nc.tensor.matmul(qp[:, ts(qb, 512)],
                                     wq_r[:, dq, ts(cc, 128)],
                                     xq_r[dq][:, ts(qb, 512)],
                                     start=(dq == 0), stop=(dq == DQC - 1))
            nc.vector.tensor_scalar_add(qT_sb[cc][:], qp[:], bq_sb[:, cc:cc + 1])

        wk_r = load_w3("wkr", wk, DQC)
        wv_r = load_w3("wvr", wv, DQC)
        keep_f = persist.tile([128, KT, HC], dt.float32, tag="keepf",
                              name="keep_f")
        nc.sync.dma_start(keep_f[:], keep)

        # ---- attention helpers ------------------------------------------
        oPs = {}
        sp_pools = [psS]          # phase C appends the extra 2-bank pool
        sp_i = [0]

        def sp_tile(name):
            # rotate score tiles over psS's 2 bufs (+ spX in phase C:
            # psS, psS, spX, psS, psS, spX, ... = 3 exps in flight)
            if len(sp_pools) == 1:
                pool = sp_pools[0]
            else:
                pool = sp_pools[0] if sp_i[0] % 3 < 2 else sp_pools[1]
            sp_i[0] += 1
            return pool.tile([128, 1024], dt.float32, tag="sp", name=name)

        def open_oP(qb, hp):
            oPs[(qb, hp)] = [
                psO.tile([128, 512], dt.float32, tag="oP", name=f"oP{qb}{hp}{h}")
                for h in range(2)
            ]

        def attn_kt(qb, hp, kt):
            kb, kti = kt // 4, kt % 4
            sp = sp_tile(f"sp{qb}{hp}{kt}")
            for h in range(2):
                nc.tensor.matmul(
                    sp[:, ts(h, 512)],
                    kT_sb[hp][kb][ts(h, 64), ts(kti, 128)],
                    qT_sb[hp][ts(h, 64), ts(qb, 512)],
                    start=True, stop=True,
                )
            pT = pT_pool.tile([128, 1024], dt.bfloat16, tag="pT",
                              name=f"pT{qb}{hp}{kt}")
            nc.scalar.activation(pT[:], sp[:], mybir.ActivationFunctionType.Exp)
            oP = oPs[(qb, hp)]
            for h in range(2):
                nc.tensor.matmul(
                    oP[h][0:65, :], va_sb[kt][:, hp * 2 + h, :], pT[:, ts(h, 512)],
                    start=(kt == 0), stop=(kt == KT - 1),
                )

        def attn_norm(qb, hp):
            oP = oPs.pop((qb, hp))
            for h in range(2):
                den = rb_pool.tile([1, 512], dt.float32, tag="den",
                                   name=f"den{qb}{hp}{h}")
                nc.vector.tensor_copy(den[:], oP[h][64:65, :])
                rdf = rb_pool.tile([1, 512], dt.float32, tag="rdf",
                                   name=f"rdf{qb}{hp}{h}")
                nc.vector.reciprocal_approx_fast(rdf[:], den[:])
                rd = rb_pool.tile([1, 512], dt.float32r, tag="rd",
                                  name=f"rd{qb}{hp}{h}")
                nc.vector.tensor_copy(rd[:], rdf[:])
                rb = sp_tile(f"rb{qb}{hp}{h}")[:, 0:512]
                nc.tensor.matmul(rb, ones128_r, rd[:], start=True, stop=True)
                rb_sb = rb_pool.tile([128, 512], dt.float32, tag="rbs",
                                     name=f"rbs{qb}{hp}{h}")
                nc.vector.tensor_copy(rb_sb[:], rb)
                nc.vector.tensor_mul(onT_sb[hp][ts(h, 64), ts(qb, 512)],
                                     oP[h][0:64, :], rb_sb[0:64, :])

        # ---- K/V projections interleaved with attention on (qb0, hp0) ---
        open_oP(0, 0)
        for kb in range(KB):
            xrs = []
            for dq in range(DQC):
                xr = xr_pool.tile([128, 512], dt.float32r, tag="xr",
                                  name=f"xk{kb}_{dq}")
                nc.sync.dma_start(xr[:], xTkv[ts(dq, 128), ts(kb, 512)])
                xrs.append(xr)
            for cc in range(2):
                kp_ps = psA.tile([128, 512], dt.float32, tag="psA",
                                 name=f"kp{kb}{cc}")
                for dq in range(DQC):
                    nc.tensor.matmul(kp_ps[:], wk_r[:, dq, ts(cc, 128)],
                                     xrs[dq][:],
                                     start=(dq == 0), stop=(dq == DQC - 1))
                nc.vector.tensor_copy(kT_sb[cc][kb][:], kp_ps[:])
            for half in range(2):
                vp = psA.tile([128, 512], dt.float32, tag="psA",
                              name=f"vp{kb}{half}")
                for dq in range(DQC):
                    for t2 in range(2):
                        t = half * 2 + t2
                        # start clears pending-write state for the whole 2KB
                        # psum bank: only its first matmul may set it
                        nc.tensor.matmul(vp[:, ts(t2, 256)],
                                         xrs[dq][:, ts(t, 128)],
                                         wv_r[:, dq, :],
                                         start=(dq == 0 and t2 == 0),
                                         stop=(dq == DQC - 1))
                for t2 in range(2):
                    t = half * 2 + t2
                    kt = kb * 4 + t
                    va = va_sb[kt]
                    src = vp[:, ts(t2, 256)].rearrange("p (h c) -> p h c", h=HC)
                    nc.vector.tensor_scalar_mul(va[:, :, 0:64], src,
                                                keep_f[:, kt, 0:1])
                    nc.vector.tensor_copy(va[:, :, 64:65], keep_f[:, kt, :])
            for t in range(4):
                attn_kt(0, 0, kb * 4 + t)

        # projections done: psA's banks become a third score buffer
        psA_cm.__exit__(None, None, None)
        spX = ctx.enter_context(tc.tile_pool(name="spX", bufs=1, space="PSUM"))
        sp_pools.append(spX)

        wo_r = load_w3("wor", wo, 2)            # [128, 2, 1024]

        attn_norm(0, 0)

        # ---- remaining attention combos ---------------------------------
        for qb, hp in [(0, 1), (1, 0), (1, 1)]:
            open_oP(qb, hp)
            for kt in range(KT):
                attn_kt(qb, hp, kt)
            attn_norm(qb, hp)

        # ---- output projection ------------------------------------------
        for qt in range(8):
            for eb in range(2):
                op = psO.tile([128, 512], dt.float32, tag="oP",
                              name=f"op{qt}_{eb}")
                for cc in range(2):
                    nc.tensor.matmul(op[:, :], onT_sb[cc][:, ts(qt, 128)],
                                     wo_r[:, cc, ts(eb, 512)],
                                     start=(cc == 0), stop=(cc == 1))
                osb = outsb_pool.tile([128, 512], dt.float32, tag="osb",
                                      name=f"osb{qt}_{eb}")
                nc.vector.tensor_copy(osb[:], op[:])
                nc.sync.dma_start(out[ts(qt, 128), ts(eb, 512)], osb[:])

        lp.__exit__(None, None, None)


def kernel(x_q, x_kv, pad_mask, Wq, bq, Wk, bk, Wv, bv, Wo, bo):
    global LAST_EXEC_NS
    x_q = np.asarray(x_q, np.float32)
    x_kv = np.asarray(x_kv, np.float32)
    pad_mask = np.asarray(pad_mask)
    Wq, bq = np.asarray(Wq, np.float32), np.asarray(bq, np.float32)
    Wk, bk = np.asarray(Wk, np.float32), np.asarray(bk, np.float32)
    Wv, bv = np.asarray(Wv, np.float32), np.asarray(bv, np.float32)
    Wo, bo = np.asarray(Wo, np.float32), np.asarray(bo, np.float32)

    if "nc" not in _cache:
        _cache["nc"] = _build()
    nc = _cache["nc"]

    Wq_s = (Wq * SCALE).astype(np.float32)
    bq_s = (bq * SCALE).astype(np.float32)

    xTq = [np.ascontiguousarray(x_q[b].T) for b in range(B)]
    xTkv = [np.ascontiguousarray(x_kv[b].T) for b in range(B)]
    keepm = []
    for b in range(B):
        k01 = (~pad_mask[b]).astype(np.float32)          # (L,) 1=keep
        k4 = np.repeat(k01[:, None], HC, axis=1)          # (L, HC)
        keepm.append(np.ascontiguousarray(
            k4.reshape(KT, 128, HC).transpose(1, 0, 2)))  # (128, KT, HC)

    in_maps = []
    for c in range(N_CORES):
        b, g = c // 4, c % 4
        hs = g * CS
        in_maps.append({
            "xTq": xTq[b],
            "xTkv": xTkv[b],
            "wq": np.ascontiguousarray(Wq_s[:, hs:hs + CS]),
            "wk": np.ascontiguousarray(Wk[:, hs:hs + CS]),
            "wv": np.ascontiguousarray(Wv[:, hs:hs + CS]),
            "wo": np.ascontiguousarray(Wo[hs:hs + CS, :]),
            "bqp": np.ascontiguousarray(
                bq_s[hs:hs + CS].reshape(2, 128).T),      # [128, 2] cc-major
            "keep": keepm[b],
        })

    res = run_bass_kernel_spmd(nc, in_maps, list(range(N_CORES)), trace=TRACE)
    LAST_EXEC_NS = res.exec_time_ns

    outp = np.zeros((B, N, D), np.float32)
    for c in range(N_CORES):
        outp[c // 4] += res.results[c]["out"]
    outp += bo + bv @ Wo
    return outp
